# revision 12
# baseline (speedup 1.0000x reference)
"""Sliding-window causal self-attention on 8 trn2 NeuronCores.

Problem: B=2, T=4096, C=512, H=8 heads (d=64), window MEMORY=256
    qkv = x @ w_attn.T ; per-head windowed-causal softmax attention ; y @ w_proj.T

Sharding: sequence-parallel. B*T = 8192 rows -> 8 chunks of 1024 queries
(4 chunks per batch element). Each core receives its 1024 query rows plus a
256-row halo of preceding tokens (zero-padded at batch starts) and computes
its output slice independently -- no collectives.

Device dataflow (per core, all layouts chosen so no on-chip transposes):
  xT [C,1280] (host-transposed), wqkT [C,3C], wpT [C,C]
  1) QKV: Q,K in head-major layout [d, t] (pairs of heads share a 128-part
     tile); V in token-major [t, (h,d)] with a ones-column appended per head.
  2) Scores: S^T[j,i] = K^T.T @ Q^T per (head, key-block) -- keys on
     partitions, so the softmax denominator comes from a matmul ones-column
     and masking bias rides the ACT exp bias (per-partition = per-key).
  3) P = exp(S/8 + kbias) * bandmask (multiplicative 0/1, post-exp, bf16).
  4) yT_aug[65, i] = V_aug.T @ P accumulated over 3 key-blocks; row 64 is the
     softmax denominator. Normalize via DVE reciprocal + gpsimd
     partition_broadcast + multiply.
  5) Projection: out[t, C] = yT.T @ wpT accumulated over 4 c-tiles.

Matmul dtypes: float32r (full-rate reduced-precision fp32) for QKV/S/proj;
bf16 for the P@V stage (P is exp output, V cast during PSUM eviction).
"""

import numpy as np
import ml_dtypes

import concourse.mybir as mybir
import concourse.tile as tile
from concourse import bacc
from concourse.bass_utils import run_bass_kernel_spmd

B, T, C = 2, 4096, 512
H, D = 8, 64
MEM = 256
NCORES = 8
TQ = 1024            # queries per core
TL = TQ + MEM        # local tokens incl halo = 1280
NQB = TQ // 128      # 8 query blocks
NJB = TL // 128      # 10 key blocks
NPAIR = 4            # head pairs
KT = C // 128        # 4 contraction tiles
F32 = mybir.dt.float32
F32R = mybir.dt.float32r
BF16 = mybir.dt.bfloat16
MASKVAL = -30000.0

_cache = {}
DEBUG = False


def _consumers(jb):
    """Query blocks consuming key block jb, and the band-mask column offset."""
    gmin = max(0, jb - 2)
    gmax = min(NQB - 1, jb)
    coff = (gmin - (jb - 2)) * 128
    return gmin, gmax, coff


def _build():
    nc = bacc.Bacc(None, target_bir_lowering=False, name="swattn")

    xT = nc.dram_tensor("xT", [C, TL], F32R, kind="ExternalInput")
    wqkT = nc.dram_tensor("wqkT", [C, 3 * C], F32R, kind="ExternalInput")
    wpT = nc.dram_tensor("wpT", [C, C], F32R, kind="ExternalInput")
    kb = nc.dram_tensor("kb", [128, NJB], F32, kind="ExternalInput")
    mask = nc.dram_tensor("mask", [128, 384], BF16, kind="ExternalInput")
    y = nc.dram_tensor("y", [TQ, C], F32, kind="ExternalOutput")
    if DEBUG:
        qdbg = nc.dram_tensor("qdbg", [128, NPAIR, TQ], F32, kind="ExternalOutput")
        kdbg = nc.dram_tensor("kdbg", [128, NPAIR, TL], F32, kind="ExternalOutput")
        vdbg = nc.dram_tensor("vdbg", [128, NJB, H, D + 1], F32, kind="ExternalOutput")
        pdbg = nc.dram_tensor("pdbg", [128, 2, 384], F32, kind="ExternalOutput")
        ydbg = nc.dram_tensor("ydbg", [128, KT, TQ], F32, kind="ExternalOutput")
        dendbg = nc.dram_tensor("dendbg", [2, TQ], F32, kind="ExternalOutput")
        sdbg = nc.dram_tensor("sdbg", [128, 2, 384], F32, kind="ExternalOutput")

    with tile.TileContext(nc) as tc:
        with tc.tile_pool(name="persist", bufs=1) as pers:
            xT_sb = pers.tile([128, KT, TL], F32R)
            wqk_sb = pers.tile([128, KT, 3 * C], F32R)
            wp_sb = pers.tile([128, KT, C], F32R)
            kb_sb = pers.tile([128, NJB], F32)
            mask_sb = pers.tile([128, 384], BF16)
            nc.sync.dma_start(xT_sb[:], xT[:].rearrange("(ko ki) t -> ki ko t", ki=128))
            nc.sync.dma_start(wqk_sb[:], wqkT[:].rearrange("(ko ki) f -> ki ko f", ki=128))
            nc.sync.dma_start(wp_sb[:], wpT[:].rearrange("(ko ki) f -> ki ko f", ki=128))
            nc.sync.dma_start(kb_sb[:], kb[:])
            nc.sync.dma_start(mask_sb[:], mask[:])

            # Q,K head-major [d, t]; pair p: partitions 0:64 = head 2p, 64:128 = head 2p+1
            qT_sb = pers.tile([128, NPAIR, TQ], F32R)
            kT_sb = pers.tile([128, NPAIR, TL], F32R)
            # V token-major with ones column: [t-block, h, 65]
            v_sb = pers.tile([128, NJB, H, D + 1], BF16)
            nc.gpsimd.memset(v_sb[:, :, :, D : D + 1], 1.0)
            # normalized attention output, c-major [c, t]
            yt_sb = pers.tile([128, KT, TQ], F32R)

            # ---- stage 1: QKV projection ----
            with tc.tile_pool(name="ps_qkv", bufs=4, space="PSUM") as ps_qkv:
                # Q: own queries only (local tokens 256:1280)
                for p in range(NPAIR):
                    for t0 in range(0, TQ, 512):
                        pq = ps_qkv.tile([128, 512], F32, tag="qkv")
                        for k in range(KT):
                            nc.tensor.matmul(
                                pq[:],
                                wqk_sb[:, k, p * 128 : (p + 1) * 128],
                                xT_sb[:, k, MEM + t0 : MEM + t0 + 512],
                                start=(k == 0), stop=(k == KT - 1),
                            )
                        nc.vector.tensor_copy(qT_sb[:, p, t0 : t0 + 512], pq[:])
                # K: all local tokens
                for p in range(NPAIR):
                    for t0 in range(0, TL, 512):
                        nt = min(512, TL - t0)
                        pk = ps_qkv.tile([128, 512], F32, tag="qkv")
                        for k in range(KT):
                            nc.tensor.matmul(
                                pk[:, :nt],
                                wqk_sb[:, k, C + p * 128 : C + (p + 1) * 128],
                                xT_sb[:, k, t0 : t0 + nt],
                                start=(k == 0), stop=(k == KT - 1),
                            )
                        nc.vector.tensor_copy(kT_sb[:, p, t0 : t0 + nt], pk[:, :nt])
                # V: token-major
                for tb in range(NJB):
                    pv = ps_qkv.tile([128, 512], F32, tag="qkv")
                    for k in range(KT):
                        nc.tensor.matmul(
                            pv[:],
                            xT_sb[:, k, tb * 128 : (tb + 1) * 128],
                            wqk_sb[:, k, 2 * C : 3 * C],
                            start=(k == 0), stop=(k == KT - 1),
                        )
                    nc.vector.tensor_copy(
                        v_sb[:, tb, :, 0:D],
                        pv[:].rearrange("t (h d) -> t h d", h=H),
                    )

            # ---- stage 2: attention, one head pair at a time ----
            with (
                tc.tile_pool(name="ps_s", bufs=2, space="PSUM") as ps_s,
                tc.tile_pool(name="ps_y", bufs=2, space="PSUM") as ps_y,
                tc.tile_pool(name="ptile", bufs=3) as ppool,
                tc.tile_pool(name="norm", bufs=2) as npool,
            ):
                for p in range(NPAIR):
                    yps = [
                        ps_y.tile([65, TQ], F32, tag="yt", name=f"yt{p}_{i}")
                        for i in range(2)
                    ]
                    for jb in range(NJB):
                        gmin, gmax, coff = _consumers(jb)
                        ncols = (gmax - gmin + 1) * 128
                        s_ps = ps_s.tile([128, 2, 512], F32, tag="s")
                        for hh in range(2):
                            nc.tensor.matmul(
                                s_ps[:, hh, :ncols],
                                kT_sb[hh * 64 : hh * 64 + 64, p, jb * 128 : (jb + 1) * 128],
                                qT_sb[hh * 64 : hh * 64 + 64, p, gmin * 128 : (gmax + 1) * 128],
                                start=True, stop=True,
                            )
                        if DEBUG and p == 0 and jb == 4:
                            sdbg_sb = pers.tile([128, 2, 384], F32, name="sdbgt")
                            nc.vector.tensor_copy(sdbg_sb[:], s_ps[:, :, :384])
                            nc.sync.dma_start(sdbg[:], sdbg_sb[:])
                        p_sb = ppool.tile([128, 2, 384], BF16, tag="p")
                        nc.scalar.activation(
                            p_sb[:, :, :ncols],
                            s_ps[:, :, :ncols],
                            mybir.ActivationFunctionType.Exp,
                            bias=kb_sb[:, jb : jb + 1],
                            scale=0.125,
                        )
                        nc.vector.tensor_tensor(
                            p_sb[:, :, :ncols],
                            p_sb[:, :, :ncols],
                            mask_sb[:, None, coff : coff + ncols].to_broadcast(
                                (128, 2, ncols)
                            ),
                            mybir.AluOpType.mult,
                        )
                        if DEBUG and p == 0 and jb == 4:
                            pdbg_sb = pers.tile([128, 2, 384], F32, name="pdbgt")
                            nc.vector.tensor_copy(pdbg_sb[:], p_sb[:])
                            nc.sync.dma_start(pdbg[:], pdbg_sb[:])
                        for hh in range(2):
                            h = 2 * p + hh
                            for g in range(gmin, gmax + 1):
                                # start=True clears has_written for the WHOLE
                                # bank, so only the first matmul touching each
                                # 512-col bank may set it; start=False on
                                # fresh bits overwrites-and-sets.
                                nc.tensor.matmul(
                                    yps[hh][:, g * 128 : (g + 1) * 128],
                                    v_sb[:, jb, h, :],
                                    p_sb[:, hh, (g - gmin) * 128 : (g - gmin + 1) * 128],
                                    start=(jb == g and g % 4 == 0),
                                    stop=(jb == g + 2 and g % 4 == 3),
                                    skip_group_check=True,
                                )
                    if DEBUG and p == 0:
                        for hh in range(2):
                            dd_sb = pers.tile([1, TQ], F32, name=f"ddt{hh}")
                            nc.vector.tensor_copy(dd_sb[:], yps[hh][64:65, :])
                            nc.sync.dma_start(dendbg[hh : hh + 1, :], dd_sb[:])
                    for hh in range(2):
                        rec = npool.tile([1, TQ], F32, tag="rec")
                        nc.vector.reciprocal(rec[:], yps[hh][64:65, :])
                        recb = npool.tile([64, TQ], F32, tag="recb")
                        nc.gpsimd.partition_broadcast(recb[:], rec[:])
                        nc.vector.tensor_tensor(
                            yt_sb[hh * 64 : hh * 64 + 64, p, :],
                            yps[hh][0:64, :],
                            recb[:],
                            mybir.AluOpType.mult,
                        )

            if DEBUG:
                nc.sync.dma_start(qdbg[:], qT_sb[:].bitcast(F32))
                nc.sync.dma_start(kdbg[:], kT_sb[:].bitcast(F32))
                vdbg_sb = pers.tile([128, NJB, H, D + 1], F32, name="vdbgt")
                nc.vector.tensor_copy(vdbg_sb[:], v_sb[:])
                nc.sync.dma_start(vdbg[:], vdbg_sb[:])
                nc.sync.dma_start(ydbg[:], yt_sb[:].bitcast(F32))

            # ---- stage 3: output projection ----
            with (
                tc.tile_pool(name="ps_o", bufs=2, space="PSUM") as ps_o,
                tc.tile_pool(name="obuf", bufs=3) as opool,
            ):
                for g in range(NQB):
                    po = ps_o.tile([128, C], F32, tag="o")
                    for k in range(KT):
                        nc.tensor.matmul(
                            po[:],
                            yt_sb[:, k, g * 128 : (g + 1) * 128],
                            wp_sb[:, k, :],
                            start=(k == 0), stop=(k == KT - 1),
                        )
                    o_sb = opool.tile([128, C], F32, tag="ob")
                    nc.vector.tensor_copy(o_sb[:], po[:])
                    nc.sync.dma_start(y[g * 128 : (g + 1) * 128, :], o_sb[:])

    nc.finalize()
    return nc


def _host_inputs(x, w_attn, w_proj):
    """Build per-core input maps (numpy only)."""
    wqkT = np.ascontiguousarray(w_attn.T.astype(np.float32))
    wpT = np.ascontiguousarray(w_proj.T.astype(np.float32))

    # band mask [128, 384]: valid iff 0 <= c - b <= MEM
    b = np.arange(128)[:, None]
    c = np.arange(384)[None, :]
    mask = ((c - b >= 0) & (c - b <= MEM)).astype(ml_dtypes.bfloat16)

    in_maps = []
    for core in range(NCORES):
        bi, ci = divmod(core, T // TQ)
        q0 = ci * TQ
        x_loc = np.zeros((TL, C), dtype=np.float32)
        lo = q0 - MEM
        src0 = max(0, lo)
        x_loc[src0 - lo :] = x[bi, src0 : q0 + TQ]
        xT_loc = np.ascontiguousarray(x_loc.T)

        kb = np.zeros((128, NJB), dtype=np.float32)
        if lo < 0:
            pad = -lo  # number of padded (invalid) leading keys
            for jb in range(NJB):
                k0 = jb * 128
                if k0 >= pad:
                    break
                kb[: min(128, pad - k0), jb] = MASKVAL

        in_maps.append(
            {"xT": xT_loc, "wqkT": wqkT, "wpT": wpT, "kb": kb, "mask": mask}
        )
    return in_maps


def kernel(x, w_attn, w_proj):
    x = np.asarray(x, dtype=np.float32)
    w_attn = np.asarray(w_attn, dtype=np.float32)
    w_proj = np.asarray(w_proj, dtype=np.float32)

    if "nc" not in _cache:
        _cache["nc"] = _build()
    nc = _cache["nc"]

    in_maps = _host_inputs(x, w_attn, w_proj)
    res = run_bass_kernel_spmd(nc, in_maps, core_ids=list(range(NCORES)))

    out = np.empty((B, T, C), dtype=np.float32)
    for core in range(NCORES):
        bi, ci = divmod(core, T // TQ)
        out[bi, ci * TQ : (ci + 1) * TQ] = res.results[core]["y"]
    return out


# revision 35
# speedup vs baseline: 453.5912x; 453.5912x over previous
"""Sliding-window causal self-attention on 8 trn2 NeuronCores.

Problem: B=2, T=4096, C=512, H=8 heads (d=64), window MEMORY=256
    qkv = x @ w_attn.T ; per-head windowed-causal softmax attention ; y @ w_proj.T

Sharding: sequence-parallel. B*T = 8192 rows -> 8 chunks of 1024 queries
(4 chunks per batch element). Each core receives its 1024 query rows plus a
256-row halo of preceding tokens (zero-padded at batch starts) and computes
its output slice independently -- no collectives.

Device dataflow (per core, all layouts chosen so no on-chip transposes):
  xT [C,1280] (host-transposed), wqkT [C,3C], wpT [C,C]
  1) QKV: Q,K in head-major layout [d, t] (pairs of heads share a 128-part
     tile); V in token-major [t, (h,d)] with a ones-column appended per head.
  2) Scores: S^T[j,i] = K^T.T @ Q^T per (head, key-block) -- keys on
     partitions, so the softmax denominator comes from a matmul ones-column
     and masking bias rides the ACT exp bias (per-partition = per-key).
  3) P = exp(S/8 + kbias) * bandmask (multiplicative 0/1, post-exp, bf16).
  4) yT_aug[65, i] = V_aug.T @ P accumulated over 3 key-blocks; row 64 is the
     softmax denominator. Normalize via DVE reciprocal + gpsimd
     partition_broadcast + multiply.
  5) Projection: out[t, C] = yT.T @ wpT accumulated over 4 c-tiles.

Matmul dtypes: float32r (full-rate reduced-precision fp32) for QKV/S/proj;
bf16 for the P@V stage (P is exp output, V cast during PSUM eviction).
"""

import numpy as np
import ml_dtypes

import concourse.mybir as mybir
import concourse.tile as tile
from concourse import bacc
from concourse.bass_utils import run_bass_kernel_spmd

B, T, C = 2, 4096, 512
H, D = 8, 64
MEM = 256
NCORES = 8
TQ = 1024            # queries per core
TL = TQ + MEM        # local tokens incl halo = 1280
NQB = TQ // 128      # 8 query blocks
NJB = TL // 128      # 10 key blocks
NPAIR = 4            # head pairs
KT = C // 128        # 4 contraction tiles
F32 = mybir.dt.float32
F32R = mybir.dt.float32r
BF16 = mybir.dt.bfloat16
MASKVAL = -30000.0

_cache = {}
DEBUG = False


def _consumers(jb):
    """Query blocks consuming key block jb, and the band-mask column offset."""
    gmin = max(0, jb - 2)
    gmax = min(NQB - 1, jb)
    coff = (gmin - (jb - 2)) * 128
    return gmin, gmax, coff


def _build():
    nc = bacc.Bacc(None, target_bir_lowering=False, name="swattn")

    xT = nc.dram_tensor("xT", [C, TL], BF16, kind="ExternalInput")
    wqkT = nc.dram_tensor("wqkT", [C, 3 * C], BF16, kind="ExternalInput")
    wpT = nc.dram_tensor("wpT", [C, C], BF16, kind="ExternalInput")
    kb = nc.dram_tensor("kb", [128, NJB], F32, kind="ExternalInput")
    mask = nc.dram_tensor("mask", [128, 2, 384], BF16, kind="ExternalInput")
    y = nc.dram_tensor("y", [TQ, C], F32, kind="ExternalOutput")
    if DEBUG:
        qdbg = nc.dram_tensor("qdbg", [128, NPAIR, TQ], F32, kind="ExternalOutput")
        kdbg = nc.dram_tensor("kdbg", [128, NPAIR, TL], F32, kind="ExternalOutput")
        vdbg = nc.dram_tensor("vdbg", [128, NJB, H, D + 1], F32, kind="ExternalOutput")
        pdbg = nc.dram_tensor("pdbg", [128, 2, 384], F32, kind="ExternalOutput")
        ydbg = nc.dram_tensor("ydbg", [128, KT, TQ], F32, kind="ExternalOutput")
        dendbg = nc.dram_tensor("dendbg", [2, TQ], F32, kind="ExternalOutput")
        sdbg = nc.dram_tensor("sdbg", [128, 2, 384], F32, kind="ExternalOutput")

    with tile.TileContext(nc) as tc:
        with tc.tile_pool(name="persist", bufs=1) as pers:
            kb_sb = pers.tile([128, NJB], F32)
            mask_sb = pers.tile([128, 2, 384], BF16)
            # per-k-tile input tiles + chunked DMAs so the first matmuls start
            # early (one monolithic DMA stalled the PE ~19us at kernel start)
            xT_r = xT[:].rearrange("(ko ki) t -> ki ko t", ki=128)
            wqk_r = wqkT[:].rearrange("(ko ki) f -> ki ko f", ki=128)
            wp_r = wpT[:].rearrange("(ko ki) f -> ki ko f", ki=128)
            xT_k = [pers.tile([128, TL], BF16, name=f"xT{k}") for k in range(KT)]
            wqk_k = [pers.tile([128, 3 * C], BF16, name=f"wqk{k}") for k in range(KT)]
            wp_k = [pers.tile([128, C], BF16, name=f"wp{k}") for k in range(KT)]
            # x chunks on the sync HWDGE queue, weights on the scalar HWDGE
            # queue -- two descriptor generators run in parallel, and each
            # QKV group unblocks as soon as its k-chunks land.
            for k in range(KT):
                nc.sync.dma_start(xT_k[k][:], xT_r[:, k, :])
                nc.scalar.dma_start(wqk_k[k][:], wqk_r[:, k, :])
            for k in range(KT):
                nc.scalar.dma_start(wp_k[k][:], wp_r[:, k, :])
            nc.sync.dma_start(kb_sb[:], kb[:])
            nc.sync.dma_start(mask_sb[:], mask[:])

            # Q,K head-major [d, t]; pair p: partitions 0:64 = head 2p, 64:128 = head 2p+1
            qT_sb = pers.tile([128, NPAIR, TQ], BF16)
            kT_sb = pers.tile([128, NPAIR, TL], BF16)
            # V token-major with ones column: [t-block, h, 65]
            v_sb = pers.tile([128, NJB, H, D + 1], BF16)
            nc.gpsimd.memset(v_sb[:, :, :, D : D + 1], 1.0)
            zz_sb = pers.tile([1, 512], BF16)
            nc.gpsimd.memset(zz_sb[:], 0.0)
            # normalized attention output, c-major [c, t]
            yt_sb = pers.tile([128, KT, TQ], BF16)

            # ---- stage 1: QKV projection ----
            # ps_a spans stages 1 and 3: the projection reuses the QKV psum
            # banks (same tag), so it never WAR-blocks on attention psum.
            with tc.tile_pool(name="ps_a", bufs=2, space="PSUM") as ps_qkv:
                # Q: own queries only (local tokens 256:1280)
                for p in range(NPAIR):
                    for t0 in range(0, TQ, 512):
                        pq = ps_qkv.tile([128, 512], F32, tag="qkv")
                        for k in range(KT):
                            nc.tensor.matmul(
                                pq[:],
                                wqk_k[k][:, p * 128 : (p + 1) * 128],
                                xT_k[k][:, MEM + t0 : MEM + t0 + 512],
                                start=(k == 0), stop=(k == KT - 1),
                            )
                        nc.vector.tensor_copy(qT_sb[:, p, t0 : t0 + 512], pq[:])
                # K: all local tokens
                for p in range(NPAIR):
                    for t0 in range(0, TL, 512):
                        nt = min(512, TL - t0)
                        pk = ps_qkv.tile([128, 512], F32, tag="qkv")
                        for k in range(KT):
                            nc.tensor.matmul(
                                pk[:, :nt],
                                wqk_k[k][:, C + p * 128 : C + (p + 1) * 128],
                                xT_k[k][:, t0 : t0 + nt],
                                start=(k == 0), stop=(k == KT - 1),
                            )
                        # K evictions on ScalarE: ACT is idle during the QKV
                        # phase, DVE is the second-busiest engine overall
                        nc.scalar.copy(kT_sb[:, p, t0 : t0 + nt], pk[:, :nt])
                # V: token-major
                for tb in range(NJB):
                    pv = ps_qkv.tile([128, 512], F32, tag="qkv")
                    for k in range(KT):
                        nc.tensor.matmul(
                            pv[:],
                            xT_k[k][:, tb * 128 : (tb + 1) * 128],
                            wqk_k[k][:, 2 * C : 3 * C],
                            start=(k == 0), stop=(k == KT - 1),
                        )
                    nc.vector.tensor_copy(
                        v_sb[:, tb, :, 0:D],
                        pv[:].rearrange("t (h d) -> t h d", h=H),
                    )

            # ---- stage 2: attention, one head pair at a time ----
            with (
                tc.tile_pool(name="ps_s", bufs=2, space="PSUM") as ps_s,
                tc.tile_pool(name="ps_y", bufs=2, space="PSUM") as ps_y,
                tc.tile_pool(name="ptile", bufs=3) as ppool,
                tc.tile_pool(name="norm", bufs=2) as npool,
            ):
                for p in range(NPAIR):
                    yps = [
                        ps_y.tile([65, TQ], F32, tag="yt", name=f"yt{p}_{i}")
                        for i in range(2)
                    ]
                    yas = [
                        npool.tile([65, TQ], F32, tag="ya", name=f"ya{p}_{i}")
                        for i in range(2)
                    ]
                    # zero-clear both banks of each yT accumulator (K=1 matmul
                    # with zero weights; start=True clears has_written)
                    for hh in range(2):
                        for bank in range(2):
                            nc.tensor.matmul(
                                yps[hh][:, bank * 512 : (bank + 1) * 512],
                                zz_sb[0:1, 0:65],
                                zz_sb[0:1, 0:512],
                                start=True, stop=False,
                                skip_group_check=True,
                            )
                    def emit_s(jb):
                        gmin, gmax, coff = _consumers(jb)
                        ncols = (gmax - gmin + 1) * 128
                        s_ps = ps_s.tile([128, 2, 512], F32, tag="s", name=f"s{p}_{jb}")
                        for hh in range(2):
                            nc.tensor.matmul(
                                s_ps[:, hh, :ncols],
                                kT_sb[hh * 64 : hh * 64 + 64, p, jb * 128 : (jb + 1) * 128],
                                qT_sb[hh * 64 : hh * 64 + 64, p, gmin * 128 : (gmax + 1) * 128],
                                start=True, stop=True,
                            )
                        return s_ps

                    def emit_rest(jb, s_ps):
                        gmin, gmax, coff = _consumers(jb)
                        ncols = (gmax - gmin + 1) * 128
                        p_sb = ppool.tile([128, 2, 384], BF16, tag="p", name=f"p{p}_{jb}")
                        nc.scalar.activation(
                            p_sb[:, :, :ncols],
                            s_ps[:, :, :ncols],
                            mybir.ActivationFunctionType.Exp,
                            bias=kb_sb[:, jb : jb + 1],
                            scale=0.125,
                        )
                        # only the two triangular 128-col blocks of the
                        # band need masking; the middle block is all-ones
                        mranges = [
                            r0 for r0 in range(0, ncols, 128)
                            if coff + r0 in (0, 256)
                        ]
                        if mranges == [0, 256]:
                            # one strided op covering both triangle blocks
                            nc.vector.tensor_tensor(
                                p_sb[:, :, :].rearrange(
                                    "p h (r c) -> p h r c", c=128
                                )[:, :, 0:3:2],
                                p_sb[:, :, :].rearrange(
                                    "p h (r c) -> p h r c", c=128
                                )[:, :, 0:3:2],
                                mask_sb[:, :, :].rearrange(
                                    "p h (r c) -> p h r c", c=128
                                )[:, :, 0:3:2],
                                mybir.AluOpType.mult,
                            )
                        else:
                            for r0 in mranges:
                                nc.vector.tensor_tensor(
                                    p_sb[:, :, r0 : r0 + 128],
                                    p_sb[:, :, r0 : r0 + 128],
                                    mask_sb[:, :, coff + r0 : coff + r0 + 128],
                                    mybir.AluOpType.mult,
                                )

                        # AV: one wide matmul per (head, key-block), split at
                        # the 512-col PSUM bank boundary. All start=False --
                        # the banks were zero-cleared by the K=1 matmuls above
                        # (start=True clears has_written for the WHOLE bank,
                        # so per-column-group starts are unusable).
                        c0 = gmin * 128
                        c1 = (gmax + 1) * 128
                        for hh in range(2):
                            h = 2 * p + hh
                            for a, b in ((c0, min(c1, 512)), (max(c0, 512), c1)):
                                if a >= b:
                                    continue
                                nc.tensor.matmul(
                                    yps[hh][:, a:b],
                                    v_sb[:, jb, h, :],
                                    p_sb[:, hh, a - c0 : b - c0],
                                    start=False,
                                    stop=(jb == NJB - 1 and b == c1),
                                    skip_group_check=True,
                                )

                        # interleaved normalization: bank A (cols 0:512) is
                        # final after jb=5, bank B after jb=9 -- evict + norm
                        # each bank as soon as it completes so the chain
                        # (DVE copy/recip, GPSIMD bcast/mult) overlaps the
                        # remaining AV matmuls of this pair / the next pair.
                        if jb == 5 or jb == NJB - 1:
                            cc = 0 if jb == 5 else 512
                            for hh in range(2):
                                ya = yas[hh]
                                nc.vector.tensor_copy(
                                    ya[:, cc : cc + 512], yps[hh][:, cc : cc + 512]
                                )
                                rec = npool.tile([1, 512], F32, tag="rec")
                                nc.vector.reciprocal(rec[:], ya[64:65, cc : cc + 512])
                                recb = npool.tile([64, 512], F32, tag="recb")
                                nc.gpsimd.partition_broadcast(recb[:], rec[:])
                                nc.gpsimd.tensor_tensor(
                                    yt_sb[hh * 64 : hh * 64 + 64, p, cc : cc + 512],
                                    ya[0:64, cc : cc + 512],
                                    recb[:],
                                    mybir.AluOpType.mult,
                                )

                    # 2-stage software pipeline: the PE stream must carry
                    # S(jb+1) BEFORE AV(jb), since engines execute their
                    # streams strictly in order -- otherwise AV(jb) stalling
                    # on exp/mask(jb) blocks the already-ready S(jb+1).
                    pending = None
                    for jb in range(NJB):
                        sp = emit_s(jb)
                        if pending is not None:
                            emit_rest(pending[0], pending[1])
                        pending = (jb, sp)
                    emit_rest(pending[0], pending[1])

            if DEBUG:
                nc.sync.dma_start(qdbg[:], qT_sb[:].bitcast(F32))
                nc.sync.dma_start(kdbg[:], kT_sb[:].bitcast(F32))
                vdbg_sb = pers.tile([128, NJB, H, D + 1], F32, name="vdbgt")
                nc.vector.tensor_copy(vdbg_sb[:], v_sb[:])
                nc.sync.dma_start(vdbg[:], vdbg_sb[:])
                nc.sync.dma_start(ydbg[:], yt_sb[:].bitcast(F32))

            # ---- stage 3: output projection ----
            with (
                tc.tile_pool(name="ps_o", bufs=2, space="PSUM") as ps_o,
                tc.tile_pool(name="obuf", bufs=3) as opool,
            ):
                for g in range(NQB):
                    po = ps_o.tile([128, C], F32, tag="o")
                    for k in range(KT):
                        nc.tensor.matmul(
                            po[:],
                            yt_sb[:, k, g * 128 : (g + 1) * 128],
                            wp_k[k][:],
                            start=(k == 0), stop=(k == KT - 1),
                        )
                    o_sb = opool.tile([128, C], F32, tag="ob")
                    nc.scalar.copy(o_sb[:], po[:])
                    nc.sync.dma_start(y[g * 128 : (g + 1) * 128, :], o_sb[:])

    nc.finalize()
    return nc


def _host_inputs(x, w_attn, w_proj):
    """Build per-core input maps (numpy only)."""
    wqkT = np.ascontiguousarray(w_attn.T.astype(ml_dtypes.bfloat16))
    wpT = np.ascontiguousarray(w_proj.T.astype(ml_dtypes.bfloat16))

    # band mask [128, 384]: valid iff 0 <= c - b <= MEM
    b = np.arange(128)[:, None]
    c = np.arange(384)[None, :]
    mask = ((c - b >= 0) & (c - b <= MEM)).astype(ml_dtypes.bfloat16)
    mask = np.ascontiguousarray(np.broadcast_to(mask[:, None, :], (128, 2, 384)))

    in_maps = []
    for core in range(NCORES):
        bi, ci = divmod(core, T // TQ)
        q0 = ci * TQ
        x_loc = np.zeros((TL, C), dtype=np.float32)
        lo = q0 - MEM
        src0 = max(0, lo)
        x_loc[src0 - lo :] = x[bi, src0 : q0 + TQ]
        xT_loc = np.ascontiguousarray(x_loc.T.astype(ml_dtypes.bfloat16))

        kb = np.zeros((128, NJB), dtype=np.float32)
        if lo < 0:
            pad = -lo  # number of padded (invalid) leading keys
            for jb in range(NJB):
                k0 = jb * 128
                if k0 >= pad:
                    break
                kb[: min(128, pad - k0), jb] = MASKVAL

        in_maps.append(
            {"xT": xT_loc, "wqkT": wqkT, "wpT": wpT, "kb": kb, "mask": mask}
        )
    return in_maps


def kernel(x, w_attn, w_proj):
    x = np.asarray(x, dtype=np.float32)
    w_attn = np.asarray(w_attn, dtype=np.float32)
    w_proj = np.asarray(w_proj, dtype=np.float32)

    if "nc" not in _cache:
        _cache["nc"] = _build()
    nc = _cache["nc"]

    in_maps = _host_inputs(x, w_attn, w_proj)
    res = run_bass_kernel_spmd(nc, in_maps, core_ids=list(range(NCORES)))

    out = np.empty((B, T, C), dtype=np.float32)
    for core in range(NCORES):
        bi, ci = divmod(core, T // TQ)
        out[bi, ci * TQ : (ci + 1) * TQ] = res.results[core]["y"]
    return out


# revision 36
# speedup vs baseline: 529.8115x; 1.1680x over previous
"""Sliding-window causal self-attention on 8 trn2 NeuronCores.

Problem: B=2, T=4096, C=512, H=8 heads (d=64), window MEMORY=256
    qkv = x @ w_attn.T ; per-head windowed-causal softmax attention ; y @ w_proj.T

Sharding: sequence-parallel. B*T = 8192 rows -> 8 chunks of 1024 queries
(4 chunks per batch element). Each core receives its 1024 query rows plus a
256-row halo of preceding tokens (zero-padded at batch starts) and computes
its output slice independently -- no collectives.

Device dataflow (per core, all layouts chosen so no on-chip transposes):
  xT [C,1280] (host-transposed), wqkT [C,3C], wpT [C,C]
  1) QKV: Q,K in head-major layout [d, t] (pairs of heads share a 128-part
     tile); V in token-major [t, (h,d)] with a ones-column appended per head.
  2) Scores: S^T[j,i] = K^T.T @ Q^T per (head, key-block) -- keys on
     partitions, so the softmax denominator comes from a matmul ones-column
     and masking bias rides the ACT exp bias (per-partition = per-key).
  3) P = exp(S/8 + kbias) * bandmask (multiplicative 0/1, post-exp, bf16).
  4) yT_aug[65, i] = V_aug.T @ P accumulated over 3 key-blocks; row 64 is the
     softmax denominator. Normalize via DVE reciprocal + gpsimd
     partition_broadcast + multiply.
  5) Projection: out[t, C] = yT.T @ wpT accumulated over 4 c-tiles.

Matmul dtypes: float32r (full-rate reduced-precision fp32) for QKV/S/proj;
bf16 for the P@V stage (P is exp output, V cast during PSUM eviction).
"""

import numpy as np
import ml_dtypes

import concourse.mybir as mybir
import concourse.tile as tile
from concourse import bacc
from concourse.bass_utils import run_bass_kernel_spmd

B, T, C = 2, 4096, 512
H, D = 8, 64
MEM = 256
NCORES = 8
TQ = 1024            # queries per core
TL = TQ + MEM        # local tokens incl halo = 1280
NQB = TQ // 128      # 8 query blocks
NJB = TL // 128      # 10 key blocks
NPAIR = 4            # head pairs
KT = C // 128        # 4 contraction tiles
F32 = mybir.dt.float32
F32R = mybir.dt.float32r
BF16 = mybir.dt.bfloat16
MASKVAL = -30000.0

_cache = {}
DEBUG = False


def _consumers(jb):
    """Query blocks consuming key block jb, and the band-mask column offset."""
    gmin = max(0, jb - 2)
    gmax = min(NQB - 1, jb)
    coff = (gmin - (jb - 2)) * 128
    return gmin, gmax, coff


def _build():
    nc = bacc.Bacc(None, target_bir_lowering=False, name="swattn")

    xT = nc.dram_tensor("xT", [C, TL], BF16, kind="ExternalInput")
    wqkT = nc.dram_tensor("wqkT", [C, 3 * C], BF16, kind="ExternalInput")
    wpT = nc.dram_tensor("wpT", [C, C], F32R, kind="ExternalInput")
    kb = nc.dram_tensor("kb", [128, NJB], F32, kind="ExternalInput")
    mask = nc.dram_tensor("mask", [128, 2, 384], BF16, kind="ExternalInput")
    y = nc.dram_tensor("y", [TQ, C], F32, kind="ExternalOutput")
    if DEBUG:
        qdbg = nc.dram_tensor("qdbg", [128, NPAIR, TQ], F32, kind="ExternalOutput")
        kdbg = nc.dram_tensor("kdbg", [128, NPAIR, TL], F32, kind="ExternalOutput")
        vdbg = nc.dram_tensor("vdbg", [128, NJB, H, D + 1], F32, kind="ExternalOutput")
        pdbg = nc.dram_tensor("pdbg", [128, 2, 384], F32, kind="ExternalOutput")
        ydbg = nc.dram_tensor("ydbg", [128, KT, TQ], F32, kind="ExternalOutput")
        dendbg = nc.dram_tensor("dendbg", [2, TQ], F32, kind="ExternalOutput")
        sdbg = nc.dram_tensor("sdbg", [128, 2, 384], F32, kind="ExternalOutput")

    with tile.TileContext(nc) as tc:
        with tc.tile_pool(name="persist", bufs=1) as pers:
            kb_sb = pers.tile([128, NJB], F32)
            mask_sb = pers.tile([128, 2, 384], BF16)
            # per-k-tile input tiles + chunked DMAs so the first matmuls start
            # early (one monolithic DMA stalled the PE ~19us at kernel start)
            xT_r = xT[:].rearrange("(ko ki) t -> ki ko t", ki=128)
            wqk_r = wqkT[:].rearrange("(ko ki) f -> ki ko f", ki=128)
            wp_r = wpT[:].rearrange("(ko ki) f -> ki ko f", ki=128)
            xT_k = [pers.tile([128, TL], BF16, name=f"xT{k}") for k in range(KT)]
            wqk_k = [pers.tile([128, 3 * C], BF16, name=f"wqk{k}") for k in range(KT)]
            wp_k = [pers.tile([128, C], F32R, name=f"wp{k}") for k in range(KT)]
            # x chunks on the sync HWDGE queue, weights on the scalar HWDGE
            # queue -- two descriptor generators run in parallel, and each
            # QKV group unblocks as soon as its k-chunks land.
            for k in range(KT):
                nc.sync.dma_start(xT_k[k][:], xT_r[:, k, :])
                nc.scalar.dma_start(wqk_k[k][:], wqk_r[:, k, :])
            for k in range(KT):
                nc.scalar.dma_start(wp_k[k][:], wp_r[:, k, :])
            nc.sync.dma_start(kb_sb[:], kb[:])
            nc.sync.dma_start(mask_sb[:], mask[:])

            # Q,K head-major [d, t]; pair p: partitions 0:64 = head 2p, 64:128 = head 2p+1
            qT_sb = pers.tile([128, NPAIR, TQ], BF16)
            kT_sb = pers.tile([128, NPAIR, TL], BF16)
            # V token-major with ones column: [t-block, h, 65]
            v_sb = pers.tile([128, NJB, H, D + 1], BF16)
            nc.gpsimd.memset(v_sb[:, :, :, D : D + 1], 1.0)
            zz_sb = pers.tile([1, 512], BF16)
            nc.gpsimd.memset(zz_sb[:], 0.0)
            # normalized attention output, c-major [c, t]
            yt_sb = pers.tile([128, KT, TQ], F32R)

            # ---- stage 1: QKV projection ----
            # ps_a spans stages 1 and 3: the projection reuses the QKV psum
            # banks (same tag), so it never WAR-blocks on attention psum.
            with tc.tile_pool(name="ps_a", bufs=2, space="PSUM") as ps_qkv:
                # Q: own queries only (local tokens 256:1280)
                for p in range(NPAIR):
                    for t0 in range(0, TQ, 512):
                        pq = ps_qkv.tile([128, 512], F32, tag="qkv")
                        for k in range(KT):
                            nc.tensor.matmul(
                                pq[:],
                                wqk_k[k][:, p * 128 : (p + 1) * 128],
                                xT_k[k][:, MEM + t0 : MEM + t0 + 512],
                                start=(k == 0), stop=(k == KT - 1),
                            )
                        nc.vector.tensor_copy(qT_sb[:, p, t0 : t0 + 512], pq[:])
                # K: all local tokens
                for p in range(NPAIR):
                    for t0 in range(0, TL, 512):
                        nt = min(512, TL - t0)
                        pk = ps_qkv.tile([128, 512], F32, tag="qkv")
                        for k in range(KT):
                            nc.tensor.matmul(
                                pk[:, :nt],
                                wqk_k[k][:, C + p * 128 : C + (p + 1) * 128],
                                xT_k[k][:, t0 : t0 + nt],
                                start=(k == 0), stop=(k == KT - 1),
                            )
                        # K evictions on ScalarE: ACT is idle during the QKV
                        # phase, DVE is the second-busiest engine overall
                        nc.scalar.copy(kT_sb[:, p, t0 : t0 + nt], pk[:, :nt])
                # V: token-major
                for tb in range(NJB):
                    pv = ps_qkv.tile([128, 512], F32, tag="qkv")
                    for k in range(KT):
                        nc.tensor.matmul(
                            pv[:],
                            xT_k[k][:, tb * 128 : (tb + 1) * 128],
                            wqk_k[k][:, 2 * C : 3 * C],
                            start=(k == 0), stop=(k == KT - 1),
                        )
                    nc.vector.tensor_copy(
                        v_sb[:, tb, :, 0:D],
                        pv[:].rearrange("t (h d) -> t h d", h=H),
                    )

            # ---- stage 2: attention, one head pair at a time ----
            with (
                tc.tile_pool(name="ps_s", bufs=2, space="PSUM") as ps_s,
                tc.tile_pool(name="ps_y", bufs=2, space="PSUM") as ps_y,
                tc.tile_pool(name="ptile", bufs=3) as ppool,
                tc.tile_pool(name="norm", bufs=2) as npool,
            ):
                for p in range(NPAIR):
                    yps = [
                        ps_y.tile([65, TQ], F32, tag="yt", name=f"yt{p}_{i}")
                        for i in range(2)
                    ]
                    yas = [
                        npool.tile([65, TQ], F32, tag="ya", name=f"ya{p}_{i}")
                        for i in range(2)
                    ]
                    # zero-clear both banks of each yT accumulator (K=1 matmul
                    # with zero weights; start=True clears has_written)
                    for hh in range(2):
                        for bank in range(2):
                            nc.tensor.matmul(
                                yps[hh][:, bank * 512 : (bank + 1) * 512],
                                zz_sb[0:1, 0:65],
                                zz_sb[0:1, 0:512],
                                start=True, stop=False,
                                skip_group_check=True,
                            )
                    def emit_s(jb):
                        gmin, gmax, coff = _consumers(jb)
                        ncols = (gmax - gmin + 1) * 128
                        s_ps = ps_s.tile([128, 2, 512], F32, tag="s", name=f"s{p}_{jb}")
                        for hh in range(2):
                            nc.tensor.matmul(
                                s_ps[:, hh, :ncols],
                                kT_sb[hh * 64 : hh * 64 + 64, p, jb * 128 : (jb + 1) * 128],
                                qT_sb[hh * 64 : hh * 64 + 64, p, gmin * 128 : (gmax + 1) * 128],
                                start=True, stop=True,
                            )
                        return s_ps

                    def emit_rest(jb, s_ps):
                        gmin, gmax, coff = _consumers(jb)
                        ncols = (gmax - gmin + 1) * 128
                        p_sb = ppool.tile([128, 2, 384], BF16, tag="p", name=f"p{p}_{jb}")
                        nc.scalar.activation(
                            p_sb[:, :, :ncols],
                            s_ps[:, :, :ncols],
                            mybir.ActivationFunctionType.Exp,
                            bias=kb_sb[:, jb : jb + 1],
                            scale=0.125,
                        )
                        # only the two triangular 128-col blocks of the
                        # band need masking; the middle block is all-ones
                        mranges = [
                            r0 for r0 in range(0, ncols, 128)
                            if coff + r0 in (0, 256)
                        ]
                        if mranges == [0, 256]:
                            # one strided op covering both triangle blocks
                            nc.vector.tensor_tensor(
                                p_sb[:, :, :].rearrange(
                                    "p h (r c) -> p h r c", c=128
                                )[:, :, 0:3:2],
                                p_sb[:, :, :].rearrange(
                                    "p h (r c) -> p h r c", c=128
                                )[:, :, 0:3:2],
                                mask_sb[:, :, :].rearrange(
                                    "p h (r c) -> p h r c", c=128
                                )[:, :, 0:3:2],
                                mybir.AluOpType.mult,
                            )
                        else:
                            for r0 in mranges:
                                nc.vector.tensor_tensor(
                                    p_sb[:, :, r0 : r0 + 128],
                                    p_sb[:, :, r0 : r0 + 128],
                                    mask_sb[:, :, coff + r0 : coff + r0 + 128],
                                    mybir.AluOpType.mult,
                                )

                        # AV: one wide matmul per (head, key-block), split at
                        # the 512-col PSUM bank boundary. All start=False --
                        # the banks were zero-cleared by the K=1 matmuls above
                        # (start=True clears has_written for the WHOLE bank,
                        # so per-column-group starts are unusable).
                        c0 = gmin * 128
                        c1 = (gmax + 1) * 128
                        for hh in range(2):
                            h = 2 * p + hh
                            for a, b in ((c0, min(c1, 512)), (max(c0, 512), c1)):
                                if a >= b:
                                    continue
                                nc.tensor.matmul(
                                    yps[hh][:, a:b],
                                    v_sb[:, jb, h, :],
                                    p_sb[:, hh, a - c0 : b - c0],
                                    start=False,
                                    stop=(jb == NJB - 1 and b == c1),
                                    skip_group_check=True,
                                )

                        # interleaved normalization: bank A (cols 0:512) is
                        # final after jb=5, bank B after jb=9 -- evict + norm
                        # each bank as soon as it completes so the chain
                        # (DVE copy/recip, GPSIMD bcast/mult) overlaps the
                        # remaining AV matmuls of this pair / the next pair.
                        if jb == 5 or jb == NJB - 1:
                            cc = 0 if jb == 5 else 512
                            for hh in range(2):
                                ya = yas[hh]
                                nc.vector.tensor_copy(
                                    ya[:, cc : cc + 512], yps[hh][:, cc : cc + 512]
                                )
                                rec = npool.tile([1, 512], F32, tag="rec")
                                nc.vector.reciprocal(rec[:], ya[64:65, cc : cc + 512])
                                recb = npool.tile([64, 512], F32, tag="recb")
                                nc.gpsimd.partition_broadcast(recb[:], rec[:])
                                nc.gpsimd.tensor_tensor(
                                    yt_sb[hh * 64 : hh * 64 + 64, p, cc : cc + 512],
                                    ya[0:64, cc : cc + 512],
                                    recb[:],
                                    mybir.AluOpType.mult,
                                )

                    # 2-stage software pipeline: the PE stream must carry
                    # S(jb+1) BEFORE AV(jb), since engines execute their
                    # streams strictly in order -- otherwise AV(jb) stalling
                    # on exp/mask(jb) blocks the already-ready S(jb+1).
                    pending = None
                    for jb in range(NJB):
                        sp = emit_s(jb)
                        if pending is not None:
                            emit_rest(pending[0], pending[1])
                        pending = (jb, sp)
                    emit_rest(pending[0], pending[1])

            if DEBUG:
                nc.sync.dma_start(qdbg[:], qT_sb[:].bitcast(F32))
                nc.sync.dma_start(kdbg[:], kT_sb[:].bitcast(F32))
                vdbg_sb = pers.tile([128, NJB, H, D + 1], F32, name="vdbgt")
                nc.vector.tensor_copy(vdbg_sb[:], v_sb[:])
                nc.sync.dma_start(vdbg[:], vdbg_sb[:])
                nc.sync.dma_start(ydbg[:], yt_sb[:].bitcast(F32))

            # ---- stage 3: output projection ----
            with (
                tc.tile_pool(name="ps_o", bufs=2, space="PSUM") as ps_o,
                tc.tile_pool(name="obuf", bufs=3) as opool,
            ):
                for g in range(NQB):
                    po = ps_o.tile([128, C], F32, tag="o")
                    for k in range(KT):
                        nc.tensor.matmul(
                            po[:],
                            yt_sb[:, k, g * 128 : (g + 1) * 128],
                            wp_k[k][:],
                            start=(k == 0), stop=(k == KT - 1),
                        )
                    o_sb = opool.tile([128, C], F32, tag="ob")
                    nc.scalar.copy(o_sb[:], po[:])
                    nc.sync.dma_start(y[g * 128 : (g + 1) * 128, :], o_sb[:])

    nc.finalize()
    return nc


def _host_inputs(x, w_attn, w_proj):
    """Build per-core input maps (numpy only)."""
    wqkT = np.ascontiguousarray(w_attn.T.astype(ml_dtypes.bfloat16))
    wpT = np.ascontiguousarray(w_proj.T.astype(np.float32))

    # band mask [128, 384]: valid iff 0 <= c - b <= MEM
    b = np.arange(128)[:, None]
    c = np.arange(384)[None, :]
    mask = ((c - b >= 0) & (c - b <= MEM)).astype(ml_dtypes.bfloat16)
    mask = np.ascontiguousarray(np.broadcast_to(mask[:, None, :], (128, 2, 384)))

    in_maps = []
    for core in range(NCORES):
        bi, ci = divmod(core, T // TQ)
        q0 = ci * TQ
        x_loc = np.zeros((TL, C), dtype=np.float32)
        lo = q0 - MEM
        src0 = max(0, lo)
        x_loc[src0 - lo :] = x[bi, src0 : q0 + TQ]
        xT_loc = np.ascontiguousarray(x_loc.T.astype(ml_dtypes.bfloat16))

        kb = np.zeros((128, NJB), dtype=np.float32)
        if lo < 0:
            pad = -lo  # number of padded (invalid) leading keys
            for jb in range(NJB):
                k0 = jb * 128
                if k0 >= pad:
                    break
                kb[: min(128, pad - k0), jb] = MASKVAL

        in_maps.append(
            {"xT": xT_loc, "wqkT": wqkT, "wpT": wpT, "kb": kb, "mask": mask}
        )
    return in_maps


def kernel(x, w_attn, w_proj):
    x = np.asarray(x, dtype=np.float32)
    w_attn = np.asarray(w_attn, dtype=np.float32)
    w_proj = np.asarray(w_proj, dtype=np.float32)

    if "nc" not in _cache:
        _cache["nc"] = _build()
    nc = _cache["nc"]

    in_maps = _host_inputs(x, w_attn, w_proj)
    res = run_bass_kernel_spmd(nc, in_maps, core_ids=list(range(NCORES)))

    out = np.empty((B, T, C), dtype=np.float32)
    for core in range(NCORES):
        bi, ci = divmod(core, T // TQ)
        out[bi, ci * TQ : (ci + 1) * TQ] = res.results[core]["y"]
    return out
